# revision 8
# baseline (speedup 1.0000x reference)
"""Trainium2 Bass kernel for nn_Blur: upfirdn2d 2x upsample with a separable
4-tap binomial FIR (depthwise), data-parallel over batch across 8 NeuronCores.

Math (per spatial dim, UP=2, KS=4, pad (1,2), taps h = [1,3,3,1]/8):
  out[2t]   = h[1]*x[t] + h[3]*x[t+1]
  out[2t+1] = h[0]*x[t] + h[2]*x[t+1]        (x[H] = 0 pad)

Per-core plan (2 images of the batch-16), groups of 4 channels with the
image-row dim H=128 on SBUF partitions:
  1. DMA in X[h=128p, c=4, w=128]
  2. TensorE: vertical filter as 2 fp32 matmuls with banded stationary
     matrices -> PSUM T_e/T_o = h[1] * (vertically filtered rows)
  3. ScalarE: s = T*(h[3]/h[1]), t = T copy into w-padded SBUF tiles
  4. VectorE: 4 adds E = t + s_shift / O = s + t_shift, writing the
     width-interleaved output rows with stride-2 APs
  5. DMA out [h, c, rowpair, 2W] -> HBM (2KB contiguous chunks)
"""
import json

import numpy as np

import concourse.bass as bass
import concourse.mybir as mybir
from concourse.bass_utils import run_bass_kernel_spmd
from concourse.tile import TileContext

f32 = mybir.dt.float32

N, C, H, W = 16, 128, 128, 128
OH, OW = 2 * H - 1, 2 * W - 1   # zero-insert upsample: (H-1)*2+1 after pad+conv
NCORES = 8
NPER = N // NCORES           # images per core
CG = 4                       # channels per group
NGRP = C // CG


# ---------------------------------------------------------------------------
# The walrus in this container supports only ONE sync-wait command per
# instruction; Tile emits up to ~3. Post-process the serialized BIR: keep one
# wait per instruction, move the rest onto inserted same-engine NoOps.
def _split_waits(bir_json: bytes) -> bytes:
    d = json.loads(bir_json)
    ctr = 0
    for fn in d["functions"]:
        for blk in fn["blocks"]:
            out = []
            for inst in blk["instructions"]:
                si = inst.get("sync_info") or {}
                ow = si.get("on_wait") or []
                if len(ow) > 1:
                    for w in ow[:-1]:
                        ctr += 1
                        out.append({
                            "debug": inst.get("debug"),
                            "engine": inst["engine"],
                            "ins": [], "outs": [],
                            "name": f"WSPL-{ctr}",
                            "opcode": "NoOp",
                            "sync_info": {"on_update": [], "on_wait": [w]},
                        })
                    si["on_wait"] = ow[-1:]
                    inst["sync_info"] = si
                out.append(inst)
            blk["instructions"] = out
    return json.dumps(d).encode()


# ---------------------------------------------------------------------------
# Walrus in this container caps sync-wait commands per CTRL instruction; the
# stock TileContext end-of-kernel drain waits on every used proc lane at once
# and fails codegen. Split it into one drain per lane.
def _install_drain_patch():
    import concourse.tile as tile_mod
    from concourse.vector_clock import ScopedClock, VectorClock

    if getattr(tile_mod.TileContext, "_drain_split_patched", False):
        return

    def _split_drain(self, tick_clock, wait_clock):
        gc = tick_clock.global_clock
        ticks = list(gc)
        nz = [i for i, t in enumerate(ticks) if t > 0]
        for i in nz or [None]:
            vec = [0] * len(ticks)
            if i is not None:
                vec[i] = ticks[i]
            d = self.nc.sync.drain()
            wait_clock.add_sem_waits(d.ins, ScopedClock({None: VectorClock(vec)}))
        self.nc.all_engine_barrier()
        assert self.sems is not None
        popped = self.nc._tile_sem_poison_stack.pop()
        assert popped is self._sem_poison
        self.nc.clear_and_free_semaphores(list(self.sems.allocated().values()))
        self.nc.all_engine_barrier()

    tile_mod.TileContext._drain_and_barrier = _split_drain
    tile_mod.TileContext._drain_split_patched = True


def _build_program(reps: int = 1):
    _install_drain_patch()
    nc = bass.Bass("TRN2")
    imgs = nc.dram_tensor("imgs", [NPER, C, H, W], f32, kind="ExternalInput")
    wmat = nc.dram_tensor("wmat", [2, H, H], f32, kind="ExternalInput")
    out = nc.dram_tensor("out", [NPER, C, OH, OW], f32, kind="ExternalOutput")
    add = mybir.AluOpType.add

    with TileContext(nc) as tc:
        import contextlib
        rep_loop = tc.For_i(0, reps, 1) if reps > 1 else contextlib.nullcontext()
        with (
            tc.tile_pool(name="cpool", bufs=1) as cpool,
            tc.tile_pool(name="xp", bufs=4) as xp,
            tc.tile_pool(name="stp", bufs=3) as stp,
            tc.tile_pool(name="pp", bufs=4, space="PSUM") as pp,
            tc.tile_pool(name="op", bufs=3) as op,
            rep_loop,
        ):
            A = cpool.tile([128, 2, H], f32)
            nc.sync.dma_start(A[:], wmat.rearrange("a k m -> k a m"))
            for n in range(NPER):
                for g in range(NGRP):
                    c0 = CG * g
                    x = xp.tile([128, CG, W], f32, tag="x")
                    nc.sync.dma_start(
                        x[:], imgs[n, c0:c0 + CG].rearrange("c h w -> h c w")
                    )
                    xf = x.rearrange("p c w -> p (c w)")
                    Te = pp.tile([128, CG * W], f32, tag="Te")
                    To = pp.tile([128, CG * W], f32, tag="To")
                    nc.tensor.matmul(Te[:], A[:, 0], xf, start=True, stop=True)
                    nc.tensor.matmul(To[:], A[:, 1], xf, start=True, stop=True)
                    # per-channel layout: [s_e | t_e | s_o | t_o], each W+1 wide
                    # with a zero pad column at local offset W.
                    st = stp.tile([128, CG, 4 * (W + 1)], f32, tag="st")
                    nc.vector.memset(st[:, :, W:4 * (W + 1):W + 1], 0.0)
                    Tev = Te.rearrange("p (c w) -> p c w", c=CG)
                    Tov = To.rearrange("p (c w) -> p c w", c=CG)
                    nc.scalar.mul(st[:, :, 0:W], Tev, 1.0 / 3.0)          # s_e
                    nc.scalar.copy(st[:, :, W + 1:2 * W + 1], Tev)        # t_e
                    nc.scalar.mul(st[:, :, 2 * W + 2:3 * W + 2], Tov, 1.0 / 3.0)  # s_o
                    nc.scalar.copy(st[:, :, 3 * W + 3:4 * W + 3], Tov)    # t_o
                    o = op.tile([128, CG, 2, OW], f32, tag="o")
                    se, te = st[:, :, 0:W + 1], st[:, :, W + 1:2 * (W + 1)]
                    so, to = st[:, :, 2 * (W + 1):3 * (W + 1)], st[:, :, 3 * (W + 1):4 * (W + 1)]
                    # even rows (phase e): even cols t+s' (128), odd cols s+t' (127)
                    nc.vector.tensor_tensor(o[:, :, 0, 0:OW:2], te[:, :, 0:W], se[:, :, 1:W + 1], add)
                    nc.vector.tensor_tensor(o[:, :, 0, 1:OW:2], se[:, :, 0:W - 1], te[:, :, 1:W], add)
                    # odd rows (phase o)
                    nc.vector.tensor_tensor(o[:, :, 1, 0:OW:2], to[:, :, 0:W], so[:, :, 1:W + 1], add)
                    nc.vector.tensor_tensor(o[:, :, 1, 1:OW:2], so[:, :, 0:W - 1], to[:, :, 1:W], add)
                    # rows 0..253 as 127 adjacent (even,odd) row pairs (510
                    # contiguous elements each), then the final even row 254
                    nc.sync.dma_start(
                        out[n, c0:c0 + CG, 0:OH - 1, :].rearrange(
                            "c (i r) w -> i c (r w)", r=2
                        ),
                        o[0:H - 1].rearrange("p c r w -> p c (r w)"),
                    )
                    nc.sync.dma_start(
                        out[n, c0:c0 + CG, OH - 1, :].unsqueeze(0),
                        o[H - 1:H, :, 0, :],
                    )

    _orig = nc.to_json_bytes
    nc.to_json_bytes = lambda: _split_waits(bytes(_orig()))
    return nc


def _make_wmat(kernel4x4: np.ndarray) -> np.ndarray:
    """Stationary matrices A_e, A_o [K=H, M=H] for the vertical polyphase
    filter, pre-scaled by the large width tap h[1] so the width pass only
    needs t = T (copy) and s = T * (h[3]/h[1])."""
    k4 = np.asarray(kernel4x4, dtype=np.float64)
    k1 = k4[0, :] / np.sqrt(k4[0, 0])  # separable factor, sums to 1
    h0, h1, h2, h3 = k1
    we = (h1, h3)   # even-phase taps
    wo = (h0, h2)   # odd-phase taps
    A = np.zeros((2, H, H), dtype=np.float64)
    idx = np.arange(H)
    A[0, idx, idx] = h1 * we[0]
    A[0, idx[:-1] + 1, idx[:-1]] = h1 * we[1]
    A[1, idx, idx] = h1 * wo[0]
    A[1, idx[:-1] + 1, idx[:-1]] = h1 * wo[1]
    return A.astype(np.float32)


_CACHE = {}


def kernel(**inputs) -> np.ndarray:
    imgs = np.asarray(inputs["imgs"], dtype=np.float32)
    kern = np.asarray(inputs["kernel"], dtype=np.float32)
    assert imgs.shape == (N, C, H, W), imgs.shape

    if "nc" not in _CACHE:
        _CACHE["nc"] = _build_program()
    nc = _CACHE["nc"]

    wmat = _make_wmat(kern)
    in_maps = [
        {"imgs": np.ascontiguousarray(imgs[i * NPER:(i + 1) * NPER]), "wmat": wmat}
        for i in range(NCORES)
    ]
    res = run_bass_kernel_spmd(nc, in_maps, core_ids=list(range(NCORES)))
    return np.concatenate([res.results[i]["out"] for i in range(NCORES)], axis=0)


# revision 25
# speedup vs baseline: 12.6550x; 12.6550x over previous
"""Trainium2 Bass kernel for nn_Blur: upfirdn2d 2x upsample with a separable
4-tap binomial FIR (depthwise), data-parallel over batch across 8 NeuronCores.

Math (per spatial dim, UP=2, KS=4, pad (1,2), taps h = [1,3,3,1]/8):
  out[2t]   = h[1]*x[t] + h[3]*x[t+1]
  out[2t+1] = h[0]*x[t] + h[2]*x[t+1]        (x[H] = 0 pad)

Per-core plan (2 images of the batch-16), groups of 4 channels with the
image-row dim H=128 on SBUF partitions:
  1. DMA in X[h=128p, c=4, w=128]
  2. TensorE: vertical filter as 2 fp32 matmuls with banded stationary
     matrices -> PSUM T_e/T_o = h[1] * (vertically filtered rows)
  3. ScalarE: s = T*(h[3]/h[1]), t = T copy into w-padded SBUF tiles
  4. VectorE: 4 adds E = t + s_shift / O = s + t_shift, writing the
     width-interleaved output rows with stride-2 APs
  5. DMA out [h, c, rowpair, 2W] -> HBM (2KB contiguous chunks)
"""
import json

import numpy as np

import concourse.bass as bass
import concourse.mybir as mybir
from concourse.tile import TileContext

f32 = mybir.dt.float32

N, C, H, W = 16, 128, 128, 128
OH, OW = 2 * H - 1, 2 * W - 1   # zero-insert upsample: (H-1)*2+1 after pad+conv
NCORES = 8
NPER = N // NCORES           # images per core
CG = 4                       # channels per group
NGRP = C // CG


# ---------------------------------------------------------------------------
# The walrus in this container supports only ONE sync-wait command per
# instruction; Tile emits up to ~3. Post-process the serialized BIR: keep one
# wait per instruction, move the rest onto inserted same-engine NoOps.
def _split_waits(bir_json: bytes) -> bytes:
    d = json.loads(bir_json)
    ctr = 0
    for fn in d["functions"]:
        for blk in fn["blocks"]:
            out = []
            for inst in blk["instructions"]:
                si = inst.get("sync_info") or {}
                ow = si.get("on_wait") or []
                if len(ow) > 1:
                    for w in ow[:-1]:
                        ctr += 1
                        out.append({
                            "debug": inst.get("debug"),
                            "engine": inst["engine"],
                            "ins": [], "outs": [],
                            "name": f"WSPL-{ctr}",
                            "opcode": "NoOp",
                            "sync_info": {"on_update": [], "on_wait": [w]},
                        })
                    si["on_wait"] = ow[-1:]
                    inst["sync_info"] = si
                out.append(inst)
            blk["instructions"] = out
    return json.dumps(d).encode()


# ---------------------------------------------------------------------------
# Walrus in this container caps sync-wait commands per CTRL instruction; the
# stock TileContext end-of-kernel drain waits on every used proc lane at once
# and fails codegen. Split it into one drain per lane.
def _install_drain_patch():
    import concourse.tile as tile_mod
    from concourse.vector_clock import ScopedClock, VectorClock

    if getattr(tile_mod.TileContext, "_drain_split_patched", False):
        return

    def _split_drain(self, tick_clock, wait_clock):
        gc = tick_clock.global_clock
        ticks = list(gc)
        nz = [i for i, t in enumerate(ticks) if t > 0]
        for i in nz or [None]:
            vec = [0] * len(ticks)
            if i is not None:
                vec[i] = ticks[i]
            d = self.nc.sync.drain()
            wait_clock.add_sem_waits(d.ins, ScopedClock({None: VectorClock(vec)}))
        self.nc.all_engine_barrier()
        assert self.sems is not None
        popped = self.nc._tile_sem_poison_stack.pop()
        assert popped is self._sem_poison
        self.nc.clear_and_free_semaphores(list(self.sems.allocated().values()))
        self.nc.all_engine_barrier()

    tile_mod.TileContext._drain_and_barrier = _split_drain
    tile_mod.TileContext._drain_split_patched = True


def _build_program(reps: int = 1, variant: str = "full", xcg: int = 64,
                   ocg: int = 16, in_eng: str = "sync", out_rings=("sync", "scalar"),
                   xbufs: int = 2, stbufs: int = 3, obufs: int = 3,
                   lean_act: bool = False):
    """variant: 'full' | 'dma_only' (no compute) | 'no_out' (no out-DMA) |
    'compute_only' (no DMAs). Non-full variants are for perf bisection only
    and produce garbage output.

    xcg: channels per input DMA (in-tile size). ocg: channels per output DMA.
    in_eng: engine for input DMAs. out_alt: alternate out-DMAs between the
    SP and ACT HWDGE rings."""
    _install_drain_patch()
    nc = bass.Bass("TRN2")
    imgs = nc.dram_tensor("imgs", [NPER, C, H, W], f32, kind="ExternalInput")
    wmat = nc.dram_tensor("wmat", [2, H, H], f32, kind="ExternalInput")
    # out rows padded to 2H and width padded to 2W so each (even,odd) row
    # pair is a 2048B-aligned contiguous chunk; host drops the pad row/col.
    out = nc.dram_tensor("out", [NPER, C, 2 * H, 2 * W], f32, kind="ExternalOutput")
    add = mybir.AluOpType.add
    do_in = variant in ("full", "dma_only", "no_out", "in_only")
    do_compute = variant in ("full", "no_out", "compute_only")
    do_out = variant in ("full", "dma_only", "out_only")
    in_dma = getattr(nc, in_eng).dma_start

    with TileContext(nc) as tc:
        import contextlib
        rep_loop = tc.For_i(0, reps, 1) if reps > 1 else contextlib.nullcontext()
        with (
            tc.tile_pool(name="cpool", bufs=1) as cpool,
            tc.tile_pool(name="xp", bufs=xbufs) as xp,
            tc.tile_pool(name="stp", bufs=stbufs) as stp,
            tc.tile_pool(name="pp", bufs=4, space="PSUM") as pp,
            tc.tile_pool(name="op", bufs=obufs) as op,
            rep_loop,
        ):
            A = cpool.tile([128, 2, H], f32)
            nc.sync.dma_start(A[:], wmat.rearrange("a k m -> k a m"))
            n_odma = 0
            for n in range(NPER):
                for g in range(NGRP):
                    c0 = CG * g
                    if g % (xcg // CG) == 0:
                        x = xp.tile([128, xcg, W], f32, tag="x")
                        if do_in:
                            in_dma(x[:], imgs[n, c0:c0 + xcg].rearrange("c h w -> h c w"))
                        elif do_compute:
                            nc.vector.memset(x[:, :, 0:1], 0.0)
                    xs = x[:, c0 % xcg:c0 % xcg + CG, :]
                    if g % (ocg // CG) == 0:
                        o = op.tile([128, ocg, 2, 2 * W], f32, tag="o")
                        if do_out and not do_compute:
                            nc.vector.memset(o[:, :, :, 0:1], 0.0)
                    os_ = o[:, c0 % ocg:c0 % ocg + CG, :, :]
                    if do_compute and lean_act:
                        if g % (ocg // CG) == 0:
                            # odd out col 255 is pad, written only here
                            nc.vector.memset(o[:, :, :, 2 * W - 1:2 * W], 0.0)
                        xf = xs.rearrange("p c w -> p (c w)")
                        Te = pp.tile([128, CG * W], f32, tag="Te")
                        To = pp.tile([128, CG * W], f32, tag="To")
                        nc.tensor.matmul(Te[:], A[:, 0], xf, start=True, stop=True)
                        nc.tensor.matmul(To[:], A[:, 1], xf, start=True, stop=True)
                        # Only s = T/3 staged in SBUF (padded); the t operand
                        # comes straight from PSUM. Layout [s_e | s_o], W+1 each.
                        st = stp.tile([128, CG, 2 * (W + 1)], f32, tag="st")
                        nc.vector.memset(st[:, :, W:2 * (W + 1):W + 1], 0.0)
                        Tev = Te.rearrange("p (c w) -> p c w", c=CG)
                        Tov = To.rearrange("p (c w) -> p c w", c=CG)
                        nc.scalar.mul(st[:, :, 0:W], Tev, 1.0 / 3.0)              # s_e
                        nc.scalar.mul(st[:, :, W + 1:2 * W + 1], Tov, 1.0 / 3.0)  # s_o
                        se, so = st[:, :, 0:W + 1], st[:, :, W + 1:2 * (W + 1)]
                        # even cols j: T[j] + s[j+1]; odd cols j<W-1: s[j] + T[j+1]
                        # (odd j=W-1 would need T[W]: lands in the dropped pad col)
                        nc.vector.tensor_tensor(os_[:, :, 0, 0:2 * W:2], Tev[:, :, 0:W], se[:, :, 1:W + 1], add)
                        nc.vector.tensor_tensor(os_[:, :, 0, 1:2 * W - 2:2], se[:, :, 0:W - 1], Tev[:, :, 1:W], add)
                        nc.vector.tensor_tensor(os_[:, :, 1, 0:2 * W:2], Tov[:, :, 0:W], so[:, :, 1:W + 1], add)
                        nc.vector.tensor_tensor(os_[:, :, 1, 1:2 * W - 2:2], so[:, :, 0:W - 1], Tov[:, :, 1:W], add)
                    elif do_compute:
                        xf = xs.rearrange("p c w -> p (c w)")
                        Te = pp.tile([128, CG * W], f32, tag="Te")
                        To = pp.tile([128, CG * W], f32, tag="To")
                        nc.tensor.matmul(Te[:], A[:, 0], xf, start=True, stop=True)
                        nc.tensor.matmul(To[:], A[:, 1], xf, start=True, stop=True)
                        # per-channel layout: [s_e | t_e | s_o | t_o], each
                        # W+1 wide with a zero pad column at local offset W.
                        st = stp.tile([128, CG, 4 * (W + 1)], f32, tag="st")
                        nc.vector.memset(st[:, :, W:4 * (W + 1):W + 1], 0.0)
                        Tev = Te.rearrange("p (c w) -> p c w", c=CG)
                        Tov = To.rearrange("p (c w) -> p c w", c=CG)
                        nc.scalar.mul(st[:, :, 0:W], Tev, 1.0 / 3.0)          # s_e
                        nc.scalar.copy(st[:, :, W + 1:2 * W + 1], Tev)        # t_e
                        nc.scalar.mul(st[:, :, 2 * W + 2:3 * W + 2], Tov, 1.0 / 3.0)  # s_o
                        nc.scalar.copy(st[:, :, 3 * W + 3:4 * W + 3], Tov)    # t_o
                        se, te = st[:, :, 0:W + 1], st[:, :, W + 1:2 * (W + 1)]
                        so, to = st[:, :, 2 * (W + 1):3 * (W + 1)], st[:, :, 3 * (W + 1):4 * (W + 1)]
                        # even rows (phase e): even cols t+s', odd cols s+t'
                        # (shifted reads cover the zero pad; out col 255 is pad)
                        nc.vector.tensor_tensor(os_[:, :, 0, 0:2 * W:2], te[:, :, 0:W], se[:, :, 1:W + 1], add)
                        nc.vector.tensor_tensor(os_[:, :, 0, 1:2 * W:2], se[:, :, 0:W], te[:, :, 1:W + 1], add)
                        # odd rows (phase o)
                        nc.vector.tensor_tensor(os_[:, :, 1, 0:2 * W:2], to[:, :, 0:W], so[:, :, 1:W + 1], add)
                        nc.vector.tensor_tensor(os_[:, :, 1, 1:2 * W:2], so[:, :, 0:W], to[:, :, 1:W + 1], add)
                    if do_out and (g + 1) % (ocg // CG) == 0:
                        # adjacent (even,odd) row pairs, 510 contiguous
                        # elements each; the final pair's odd row is the pad.
                        oc0 = c0 + CG - ocg
                        eng = getattr(nc, out_rings[n_odma % len(out_rings)])
                        n_odma += 1
                        eng.dma_start(
                            out[n, oc0:oc0 + ocg].rearrange("c (i r) w -> i c (r w)", r=2),
                            o.rearrange("p c r w -> p c (r w)"),
                        )

    _orig = nc.to_json_bytes
    nc.to_json_bytes = lambda: _split_waits(bytes(_orig()))
    return nc


def _make_wmat(kernel4x4: np.ndarray) -> np.ndarray:
    """Stationary matrices A_e, A_o [K=H, M=H] for the vertical polyphase
    filter, pre-scaled by the large width tap h[1] so the width pass only
    needs t = T (copy) and s = T * (h[3]/h[1])."""
    k4 = np.asarray(kernel4x4, dtype=np.float64)
    k1 = k4[0, :] / np.sqrt(k4[0, 0])  # separable factor, sums to 1
    h0, h1, h2, h3 = k1
    we = (h1, h3)   # even-phase taps
    wo = (h0, h2)   # odd-phase taps
    A = np.zeros((2, H, H), dtype=np.float64)
    idx = np.arange(H)
    A[0, idx, idx] = h1 * we[0]
    A[0, idx[:-1] + 1, idx[:-1]] = h1 * we[1]
    A[1, idx, idx] = h1 * wo[0]
    A[1, idx[:-1] + 1, idx[:-1]] = h1 * wo[1]
    return A.astype(np.float32)


_CACHE = {}


def _get_exec():
    """Compile the bass program and wrap it in a cached sharded jit callable
    (mirrors bass2jax.run_bass_via_pjrt's multi-core path, minus donation so
    the callable is reusable)."""
    if "fn" in _CACHE:
        return _CACHE["fn"]
    import jax
    from jax.sharding import Mesh, PartitionSpec, NamedSharding
    from jax.experimental.shard_map import shard_map
    from concourse import bass2jax

    nc = _build_program()
    bass2jax.install_neuronx_cc_hook()
    partition_name = nc.partition_id_tensor.name if nc.partition_id_tensor else None

    in_names, out_names, out_avals = [], [], []
    for alloc in nc.m.functions[0].allocations:
        if not isinstance(alloc, mybir.MemoryLocationSet):
            continue
        name = alloc.memorylocations[0].name
        if alloc.kind == "ExternalInput":
            if name != partition_name:
                in_names.append(name)
        elif alloc.kind == "ExternalOutput":
            out_names.append(name)
            out_avals.append(jax.core.ShapedArray(
                tuple(alloc.tensor_shape), mybir.dt.np(alloc.dtype)))
    all_in_names = list(in_names) + list(out_names)
    if partition_name is not None:
        all_in_names.append(partition_name)
    n_params = len(in_names)
    n_outs = len(out_avals)

    def _body(*args):
        operands = list(args)
        if partition_name is not None:
            operands.append(bass2jax.partition_id_tensor())
        return tuple(bass2jax._bass_exec_p.bind(
            *operands,
            out_avals=tuple(out_avals),
            in_names=tuple(all_in_names),
            out_names=tuple(out_names),
            lowering_input_output_aliases=(),
            sim_require_finite=True,
            sim_require_nnan=True,
            nc=nc,
        ))

    devices = jax.devices()[:NCORES]
    mesh = Mesh(np.asarray(devices), ("core",))
    fn = jax.jit(
        shard_map(_body, mesh=mesh,
                  in_specs=(PartitionSpec("core"),) * (n_params + n_outs),
                  out_specs=(PartitionSpec("core"),) * n_outs,
                  check_rep=False),
        keep_unused=True,
    )
    sharding = NamedSharding(mesh, PartitionSpec("core"))
    zeros = [np.zeros((NCORES * a.shape[0], *a.shape[1:]), a.dtype) for a in out_avals]
    _CACHE["fn"] = (fn, in_names, sharding, zeros)
    return _CACHE["fn"]


def kernel(**inputs) -> np.ndarray:
    import jax
    imgs = np.ascontiguousarray(np.asarray(inputs["imgs"], dtype=np.float32))
    kern = np.asarray(inputs["kernel"], dtype=np.float32)
    assert imgs.shape == (N, C, H, W), imgs.shape

    fn, in_names, sharding, zeros = _get_exec()
    wmat = _make_wmat(kern)
    by_name = {
        "imgs": imgs,  # batch is already the leading axis: shard_map splits it
        "wmat": np.concatenate([wmat] * NCORES, axis=0),
    }
    args = [jax.device_put(by_name[nm], sharding) for nm in in_names]
    zargs = [jax.device_put(z, sharding) for z in zeros]
    outs = fn(*args, *zargs)
    full = np.asarray(outs[0]).reshape(N, C, 2 * H, 2 * W)
    return np.ascontiguousarray(full[:, :, :OH, :OW])
